# revision 1
# baseline (speedup 1.0000x reference)
"""Multi-head causal self-attention (B=8, S=1024, D=768, H=12) on 8 TRN2
NeuronCores, data-parallel over the batch dimension (one batch element per
core).

Per-core pipeline (all matmuls in float32r — fp32 storage, ~11-bit-mantissa
matmul datapath, 1 cycle/row):
  1. x [S,D] -> xT [D,S] via PE transposes (identity matmul)
  2. qkT [1536,S] = (Wqk)^T xT   (Q,K in transposed layout, head dim on
     partitions);  v [S,768] = x Wv in natural layout with an extra ones
     column per head (for softmax denominators)
  3. per head: scoresT[kp,qp] = k q^T; exp via ACT straight out of PSUM with
     the 1/sqrt(hd) scale folded in (scores are O(1) here so no max
     subtraction is needed); causal handled by skipping dead blocks,
     shrinking matmul N to the live range, and one triangular mask multiply
     per diagonal block;  attnT[hd+1,qp] = [v|1]^T expT gives the attention
     output and the softmax denominator in one accumulation;  normalize via
     vector reciprocal + gpsimd partition_broadcast + vector multiply
  4. out[s,e] = attnT^T Wout, evacuated to DRAM;  biases are all-zero in
     this problem (asserted) and bout is added on the host
"""

import sys

import numpy as np

for _p in ("/opt/trn_rl_repo", "/root/.axon_site/_ro/trn_rl_repo"):
    if _p not in sys.path:
        sys.path.append(_p)

import concourse.mybir as mybir  # noqa: E402
import concourse.tile as tile  # noqa: E402
from concourse import bacc  # noqa: E402
from concourse.bass_utils import run_bass_kernel_spmd  # noqa: E402

F32 = mybir.dt.float32
F32R = mybir.dt.float32r

B, S, D = 8, 1024, 768
H, HD = 12, 64
ND3 = 3 * D
SCALE = 0.125  # 1/sqrt(64)
P = 128
NT_QK = 12        # 1536 / 128 q+k row tiles
DT = 6            # 768 / 128 contraction tiles
ST = 8            # 1024 / 128 sequence tiles
CH = 2            # 1024 / 512 sequence chunks
VW = 65           # per-head v width incl. ones column


def round_f32r(x: np.ndarray) -> np.ndarray:
    """Round fp32 to the fp32r grid (11 mantissa bits) with round-to-nearest-
    even, so the on-chip truncating read sees RTNE-rounded values."""
    u = np.ascontiguousarray(x, dtype=np.float32).view(np.uint32)
    lsb = (u >> np.uint32(12)) & np.uint32(1)
    r = (u + np.uint32(0x7FF) + lsb) & np.uint32(0xFFFFF000)
    return r.view(np.float32)


def build(ctx, tc: tile.TileContext, aps: dict):
    nc = tc.nc
    hs, wqkv, wout, ident, tri, out_d = (
        aps["hs"], aps["wqkv"], aps["wout"], aps["ident"], aps["tri"], aps["out"])

    pool_p = ctx.enter_context(tc.tile_pool(name="persist", bufs=1))
    pool_sh = ctx.enter_context(tc.tile_pool(name="share", bufs=1))
    pool_x = ctx.enter_context(tc.tile_pool(name="xslot", bufs=1))
    pool_s = ctx.enter_context(tc.tile_pool(name="small", bufs=2))
    ps_a = ctx.enter_context(tc.tile_pool(name="psA", bufs=2, space="PSUM"))
    ps_b = ctx.enter_context(tc.tile_pool(name="psB", bufs=4, space="PSUM"))

    # ---- persistent SBUF tensors ----
    x_nat = pool_x.tile([P, ST, D], F32R, tag="xslot")
    wqk_sb = pool_sh.tile([P, DT, 2 * D], F32R, tag="shareA")
    wv_sb = pool_sh.tile([P, DT, D], F32R, tag="shareB")
    wout_sb = pool_p.tile([P, DT, D], F32R, tag="wout")
    xT = pool_p.tile([P, DT, S], F32R, tag="xT")
    qkT = pool_p.tile([P, NT_QK, S], F32R, tag="qkT")
    v_buf = pool_p.tile([P, ST, H * VW], F32R, tag="vbuf")
    ident_sb = pool_p.tile([P, P], F32R, tag="ident")
    tri_sb = pool_p.tile([P, P], F32, tag="tri")

    # ---- input DMAs ----
    nc.sync.dma_start(ident_sb[:], ident)
    nc.sync.dma_start(tri_sb[:], tri)
    for st in range(ST):
        for half in range(2):
            nc.sync.dma_start(
                x_nat[:, st, half * (D // 2):(half + 1) * (D // 2)],
                hs[st * P:(st + 1) * P, half * (D // 2):(half + 1) * (D // 2)])
    for dt in range(DT):
        for half in range(2):
            nc.sync.dma_start(
                wqk_sb[:, dt, half * D:(half + 1) * D],
                wqkv[dt * P:(dt + 1) * P, half * D:(half + 1) * D])
    for dt in range(DT):
        nc.sync.dma_start(wv_sb[:, dt, :],
                          wqkv[dt * P:(dt + 1) * P, 2 * D:ND3])
    for dt in range(DT):
        nc.sync.dma_start(wout_sb[:, dt, :], wout[dt * P:(dt + 1) * P, :])

    # ones columns of v_buf (col 64 of each per-head 65-wide slab)
    vb_ones = v_buf.rearrange("p s (h x) -> p s h x", x=VW)[:, :, :, 64]
    nc.vector.memset(vb_ones.bitcast(F32), 1.0)

    # ---- phase 1: x -> xT via PE transposes ----
    # two transposes per psum tile -> one strided evac for both (halves the
    # per-call ACT/DVE overhead), alternating evac engines
    for st in range(ST):
        for dp in range(DT // 2):
            dt = 2 * dp
            pt = ps_a.tile([P, 2, 512], F32, tag="psA")
            nc.tensor.transpose(
                pt[:, 0, 0:P].bitcast(F32R),
                x_nat[:, st, dt * P:(dt + 1) * P], ident_sb[:])
            nc.tensor.transpose(
                pt[:, 1, 0:P].bitcast(F32R),
                x_nat[:, st, (dt + 1) * P:(dt + 2) * P], ident_sb[:])
            dst = xT[:, dt:dt + 2, st * P:(st + 1) * P]
            if (st * 3 + dp) % 2 == 0:
                nc.scalar.copy(dst, pt[:, :, 0:P])
            else:
                nc.vector.tensor_copy(dst, pt[:, :, 0:P])

    # ---- phase 2: QKV projections ----
    # q,k transposed: qkT[nt*128+m, s] ; emit nt order pairs (q0,k0,q1,k1...)
    nt_order = [x for pair in zip(range(6), range(6, 12)) for x in pair]
    for c in range(CH):
        for ni, nt in enumerate(nt_order):
            if ni % 3 == 2:
                pq2 = ps_a.tile([P, 2, 512], F32, tag="psA", name="pqa")
                pq = pq2[:, 0]
            else:
                pq = ps_b.tile([P, 512], F32, tag="psB", name="pqb")
            for dt in range(DT):
                nc.tensor.matmul(
                    pq[:],
                    wqk_sb[:, dt, nt * P:(nt + 1) * P],
                    xT[:, dt, c * 512:(c + 1) * 512],
                    start=(dt == 0), stop=(dt == DT - 1))
            nc.scalar.copy(qkT[:, nt, c * 512:(c + 1) * 512], pq[:])
        # v rows for the 4 sequence tiles of this chunk, natural layout
        for st in range(c * 4, c * 4 + 4):
            for vc, (n0, nw) in enumerate(((0, 512), (512, 256))):
                pv = ps_b.tile([P, 512], F32, tag="psB")
                for dt in range(DT):
                    nc.tensor.matmul(
                        pv[:, 0:nw],
                        xT[:, dt, st * P:(st + 1) * P],
                        wv_sb[:, dt, n0:n0 + nw],
                        start=(dt == 0), stop=(dt == DT - 1))
                dst = v_buf.rearrange("p s (h x) -> p s h x", x=VW)[
                    :, st, vc * 8:vc * 8 + nw // HD, 0:HD]
                nc.scalar.copy(
                    dst, pv[:, 0:nw].rearrange("p (h x) -> p h x", x=HD))

    # ---- phase 3: attention, one head at a time ----
    # expT: single-buffered in the dead x_nat slot (region-level tracking
    # keeps exp/PV pipelined); attnT in the dead wv slot. Neither aliases
    # wqk, so attention overlaps the tail of the QKV phase.
    expT0 = pool_x.tile([P, ST, 512], F32R, tag="xslot")
    expT1 = pool_sh.tile([P, ST, 512], F32R, tag="shareA")
    attnT = pool_sh.tile([P, DT, S], F32R, tag="shareB")
    for h in range(H):
        r0 = 64 * (h % 2)
        qt, kt = h // 2, 6 + h // 2
        for c in range(CH):
            expT = expT0 if (h * CH + c) % 2 == 0 else expT1
            nk = 4 * c + 4                      # live kp tiles: 0 .. nk-1
            for kg in range(nk // 2):
                k0, k1 = 2 * kg, 2 * kg + 1
                s0 = max(0, k0 - 4 * c) * P
                s1 = max(0, k1 - 4 * c) * P
                sc = ps_a.tile([P, 2, 512], F32, tag="psA")
                for i, (k, sk) in enumerate(((k0, s0), (k1, s1))):
                    nc.tensor.matmul(
                        sc[:, i, sk:512],
                        qkT[r0:r0 + HD, kt, k * P:(k + 1) * P],
                        qkT[r0:r0 + HD, qt, c * 512 + sk:(c + 1) * 512],
                        start=True, stop=True)
                nc.scalar.activation(
                    expT[:, k0:k0 + 2, s0:512], sc[:, :, s0:512],
                    mybir.ActivationFunctionType.Exp, scale=SCALE)
                for k, sk in ((k0, s0), (k1, s1)):
                    d = k - 4 * c
                    if 0 <= d <= 3:             # diagonal block: mask
                        sl = expT[:, k, d * P:(d + 1) * P]
                        nc.vector.tensor_tensor(
                            sl, sl.bitcast(F32), tri_sb[:],
                            mybir.AluOpType.mult)
            # PV: attnT_unnorm [65, 512] with row 64 = softmax denominator
            pv = ps_b.tile([P, 512], F32, tag="psB")
            for k in range(nk):
                sk = max(0, k - 4 * c) * P
                nc.tensor.matmul(
                    pv[0:VW, sk:512],
                    v_buf[:, k, h * VW:(h + 1) * VW],
                    expT[:, k, sk:512],
                    start=(k == 0), stop=(k == nk - 1))
            rcp = pool_s.tile([1, 512], F32, tag="dn")
            nc.vector.reciprocal(rcp[:], pv[64:65, :])
            rep_sb = pool_s.tile([HD, 512], F32, tag="repsb")
            nc.gpsimd.partition_broadcast(rep_sb[:], rcp[:])
            nc.vector.tensor_tensor(
                attnT[r0:r0 + HD, h // 2, c * 512:(c + 1) * 512],
                pv[0:HD, :], rep_sb[:], mybir.AluOpType.mult)

    # ---- phase 4: output projection (staging tiles in the dead wqk slot,
    # manually alternated per s-tile) ----
    out2_all = pool_sh.tile([P, 2, D], F32, tag="shareA")
    for st in range(ST):
        o2 = out2_all[:, st % 2]
        for n0, nw in ((0, 512), (512, 256)):
            po = ps_b.tile([P, 512], F32, tag="psB")
            for dt in range(DT):
                nc.tensor.matmul(
                    po[:, 0:nw],
                    attnT[:, dt, st * P:(st + 1) * P],
                    wout_sb[:, dt, n0:n0 + nw],
                    start=(dt == 0), stop=(dt == DT - 1))
            nc.vector.tensor_copy(o2[:, n0:n0 + nw], po[:, 0:nw])
            nc.sync.dma_start(out_d[st * P:(st + 1) * P, n0:n0 + nw],
                              o2[:, n0:n0 + nw])


def build_module():
    nc = bacc.Bacc("TRN2", target_bir_lowering=False, debug=False)
    aps = {
        "hs": nc.dram_tensor("hs", [S, D], F32R, kind="ExternalInput").ap(),
        "wqkv": nc.dram_tensor("wqkv", [D, ND3], F32R,
                               kind="ExternalInput").ap(),
        "wout": nc.dram_tensor("wout", [D, D], F32R,
                               kind="ExternalInput").ap(),
        "ident": nc.dram_tensor("ident", [P, P], F32R,
                                kind="ExternalInput").ap(),
        "tri": nc.dram_tensor("tri", [P, P], F32, kind="ExternalInput").ap(),
        "out": nc.dram_tensor("out", [S, D], F32, kind="ExternalOutput").ap(),
    }
    from contextlib import ExitStack
    with tile.TileContext(nc) as tc, ExitStack() as ctx:
        build(ctx, tc, aps)
    nc.compile()
    return nc


def kernel(hidden_states, Wqkv, bqkv, Wout, bout, _run_kwargs=None):
    hidden_states = np.asarray(hidden_states, dtype=np.float32)
    Wqkv = np.asarray(Wqkv, dtype=np.float32)
    bqkv = np.asarray(bqkv, dtype=np.float32)
    Wout = np.asarray(Wout, dtype=np.float32)
    bout = np.asarray(bout, dtype=np.float32)
    assert not np.any(bqkv), "nonzero qkv bias not supported by this kernel"

    nc = build_module()

    wqkv_r = round_f32r(Wqkv)
    wout_r = round_f32r(Wout)
    ident = np.eye(P, dtype=np.float32)
    tri = np.triu(np.ones((P, P), dtype=np.float32))
    in_maps = [
        {
            "hs": round_f32r(hidden_states[b]),
            "wqkv": wqkv_r,
            "wout": wout_r,
            "ident": ident,
            "tri": tri,
        }
        for b in range(B)
    ]
    res = run_bass_kernel_spmd(nc, in_maps, core_ids=list(range(B)),
                               **(_run_kwargs or {}))
    out = np.stack([res.results[b]["out"] for b in range(B)])
    if np.any(bout):
        out = out + bout
    kernel.last_results = res
    return out.astype(np.float32)



# revision 16
# speedup vs baseline: 1.2647x; 1.2647x over previous
"""Multi-head causal self-attention (B=8, S=1024, D=768, H=12) on 8 TRN2
NeuronCores, data-parallel over the batch (one batch element per core).

Per-core pipeline (dtype-tiered for the TRN2 cost model):
  1. x is sent twice from the host: bf16 (for V) and fp8e4*8 (for Q,K).
     PE transposes build xT in both dtypes.
  2. Q,K projection runs fp8 DoubleRow (dt-pairs -> 256-deep contraction at
     0.5 cyc/row). Wqkv's Q,K columns are PERMUTED on the host so each
     128-row PSUM tile holds [4 heads x 32 hd-dims]; the evacuated fp8 qkT
     planes then feed DoubleRow scores directly: head h uses partitions
     32*(h%4).. with the two 32-dim slabs as the DoubleRow pair.
  3. Scores (fp8 DR, transposed layout [k,q]) -> exp on ACT (scale folds the
     1/sqrt(hd) and the fp8 scaling) -> bf16 expT -> causal mask multiply
     (one DVE op per head-chunk over a prebuilt [tri|1|1|tri] pattern).
  4. PV runs in NATURAL orientation: out[q_part, head, hd+1] accumulated
     over k-tiles with v|ones moving operand; the ones column gives the
     softmax denominator per (q-partition, head), so normalization is a
     single DVE divide with a stride-0 broadcast AP, fused with the PSUM
     evacuation into attn_nat.
  5. attn_nat -> attnT via PE transposes; out-projection in bf16; bf16
     result DMA'd out and upcast on the host. Attention runs q-chunk-major
     so out-projection overlaps later chunks' exp/PV.
"""

import sys

import numpy as np

for _p in ("/opt/trn_rl_repo", "/root/.axon_site/_ro/trn_rl_repo"):
    if _p not in sys.path:
        sys.path.append(_p)

import concourse.mybir as mybir  # noqa: E402
import concourse.tile as tile  # noqa: E402
from concourse import bacc  # noqa: E402
from concourse.bass_utils import run_bass_kernel_spmd  # noqa: E402

F32 = mybir.dt.float32
BF16 = mybir.dt.bfloat16
F8 = mybir.dt.float8e4

B, S, D = 8, 1024, 768
H, HD = 12, 64
P = 128
DT = 6            # 768 / 128 contraction tiles
ST = 8            # 1024 / 128 sequence tiles
QCW = 256         # q-chunk width for attention
NQC = S // QCW    # 4
WS = 128.0        # host scale baked into wqk8 (lifts W out of fp8 subnormals)
EXP_SCALE = 0.125 / (WS * WS)
USE_DIVIDE = True


def build(ctx, tc: tile.TileContext, aps: dict):
    nc = tc.nc
    xb_d, wqk_d, wv_d, wo_d, id_d, tri_d, out_d = (
        aps["xb"], aps["wqk8"], aps["wv"], aps["wo"],
        aps["identb"], aps["trib4"], aps["out"])

    pool_p = ctx.enter_context(tc.tile_pool(name="persist", bufs=1))
    pool_e = ctx.enter_context(tc.tile_pool(name="expT", bufs=6))
    pool_o = ctx.enter_context(tc.tile_pool(name="ostage", bufs=2))
    ps_sc = ctx.enter_context(tc.tile_pool(name="psSC", bufs=2, space="PSUM"))
    ps_pv = ctx.enter_context(tc.tile_pool(name="psPV", bufs=2, space="PSUM"))
    ps_pj = ctx.enter_context(tc.tile_pool(name="psPJ", bufs=2, space="PSUM"))

    # ---- persistent SBUF ----
    x_b = pool_p.tile([P, ST, D], BF16, tag="xb")
    xT8 = pool_p.tile([P, DT, S], F8, tag="xT8")
    xTb = pool_p.tile([P, DT, S], BF16, tag="xTb")
    wqk_sb = pool_p.tile([P, DT, 2 * D], F8, tag="wqk")
    wv_sb = pool_p.tile([P, DT, D], BF16, tag="wv")
    wo_sb = pool_p.tile([P, DT, D], BF16, tag="wo")
    qkT = pool_p.tile([P, 2 * DT, S], F8, tag="qkT")     # planes 0-5 q, 6-11 k
    v_buf = pool_p.tile([P, ST, H, HD + 1], BF16, tag="vbuf")
    attn_nat = pool_p.tile([P, ST, H, HD], BF16, tag="anat")
    attnT = pool_p.tile([P, DT, S], BF16, tag="attnT")
    ident_sb = pool_p.tile([P, P], BF16, tag="ident")
    tri4_sb = pool_p.tile([P, 4 * P], BF16, tag="tri4")
    scratch = pool_p.tile([1, 8], BF16, tag="scr")

    # ---- input DMAs (order = arrival order on the shared DMA device) ----
    nc.sync.dma_start(ident_sb[:], id_d)
    nc.sync.dma_start(tri4_sb[:], tri_d)
    xb_r = xb_d.rearrange("(t p) d -> p t d", p=P)
    nc.sync.dma_start(x_b[:, 0:4], xb_r[:, 0:4])
    nc.sync.dma_start(wqk_sb[:], wqk_d.rearrange("(t p) d -> p t d", p=P))
    nc.sync.dma_start(x_b[:, 4:8], xb_r[:, 4:8])
    nc.sync.dma_start(wv_sb[:], wv_d.rearrange("(t p) d -> p t d", p=P))
    nc.sync.dma_start(wo_sb[:], wo_d.rearrange("(t p) d -> p t d", p=P))

    # preload the exp table set while DMAs stream
    nc.scalar.activation(scratch[0:1, 0:1], ident_sb[0:1, 0:1],
                         mybir.ActivationFunctionType.Exp, scale=1.0)
    # ones columns of v_buf (softmax denominators ride the PV matmul)
    nc.gpsimd.memset(v_buf[:, :, :, HD], 1.0)

    # ---- bf16 transposes: xb -> xTb (DVE evac) + xT8 (fp8 convert, ACT) ----
    for st in range(ST):
        pjt = ps_pj.tile([P, 512], F32, tag="pj")
        bfv = pjt[:].bitcast(BF16).rearrange("p (n c) -> p n c", c=P)  # 8 slots
        for dt in range(DT):
            nc.tensor.transpose(
                bfv[:, dt], x_b[:, st, dt * P:(dt + 1) * P], ident_sb[:])
        src = pjt[:].bitcast(BF16)[:, 0:6 * P].rearrange("p (d x) -> p d x", x=P)
        nc.scalar.copy(xT8[:, :, st * P:(st + 1) * P], src)
        nc.vector.tensor_copy(xTb[:, :, st * P:(st + 1) * P], src)

    # ---- QK projection (fp8 DoubleRow over dt-pairs) ----
    plane_order = [0, 6, 1, 7, 2, 8, 3, 9, 4, 10, 5, 11]

    def qk_chunk(c):
        for pl in plane_order:
            pq = ps_pj.tile([P, 512], F32, tag="pj")
            for i in range(3):
                nc.tensor.matmul(
                    pq[:],
                    wqk_sb[:, 2 * i:2 * i + 2, pl * P:(pl + 1) * P],
                    xT8[:, 2 * i:2 * i + 2, c * 512:(c + 1) * 512],
                    start=(i == 0), stop=(i == 2),
                    perf_mode=mybir.MatmulPerfMode.DoubleRow)
            nc.vector.tensor_copy(qkT[:, pl, c * 512:(c + 1) * 512], pq[:])

    qk_chunk(0)
    qk_chunk(1)

    # ---- V projection (bf16) ----
    for st in range(ST):
        for n0, nw in ((0, 512), (512, 256)):
            pv_ = ps_pj.tile([P, 512], F32, tag="pj")
            for dt in range(DT):
                nc.tensor.matmul(
                    pv_[:, 0:nw],
                    xTb[:, dt, st * P:(st + 1) * P],
                    wv_sb[:, dt, n0:n0 + nw],
                    start=(dt == 0), stop=(dt == DT - 1))
            nc.vector.tensor_copy(
                v_buf[:, st, n0 // HD:(n0 + nw) // HD, 0:HD],
                pv_[:, 0:nw].rearrange("p (h x) -> p h x", x=HD))

    # ---- attention, q-chunk-major ----
    for qc in range(NQC):
        nlive = 2 * qc + 2
        exp_tiles = {}
        for r in range(4):                   # head triples
            for h in range(3 * r, 3 * r + 3):
                g, j = h // 4, h % 4
                et = pool_e.tile([P, ST, QCW], BF16, tag="expT")
                exp_tiles[h] = et
                for t0 in range(0, nlive, 4):
                    t1 = min(t0 + 4, nlive)
                    sct = ps_sc.tile([P, 4, QCW], F32, tag="sc")
                    for kt in range(t0, t1):
                        sk = P if kt == 2 * qc + 1 else 0
                        nc.tensor.matmul(
                            sct[:, kt - t0, sk:QCW],
                            qkT[32 * j:32 * j + 32, 6 + 2 * g:8 + 2 * g,
                                kt * P:(kt + 1) * P],
                            qkT[32 * j:32 * j + 32, 2 * g:2 * g + 2,
                                qc * QCW + sk:(qc + 1) * QCW],
                            start=True, stop=True,
                            perf_mode=mybir.MatmulPerfMode.DoubleRow,
                            tile_position=(32 * j, 0))
                    nc.scalar.activation(
                        et[:, t0:t1, :], sct[:, 0:t1 - t0, :],
                        mybir.ActivationFunctionType.Exp, scale=EXP_SCALE)
                # causal mask: [tri|1|1|tri] over the two diagonal planes
                dm = et[:].rearrange("p a b -> p (a b)")[
                    :, 2 * qc * QCW:2 * qc * QCW + 512]
                nc.vector.tensor_tensor(dm, dm, tri4_sb[:],
                                        mybir.AluOpType.mult)
            # PV for this triple (natural orientation, ones col = denom)
            pvt = ps_pv.tile([P, 2, QCW], F32, tag="pv")
            for qbl in range(2):
                qb = 2 * qc + qbl
                for hl in range(3):
                    h = 3 * r + hl
                    for kt in range(qb + 1):
                        nc.tensor.matmul(
                            pvt[:, qbl, hl * 65:hl * 65 + 65],
                            exp_tiles[h][:, kt, qbl * P:(qbl + 1) * P],
                            v_buf[:, kt, h, :],
                            start=(kt == 0), stop=(kt == qb))
            # fused evacuate + normalize: recip of the denominators (PSUM ->
            # small SBUF tile), then one multiply with a broadcast AP
            pvr = pvt[:, :, 0:195].rearrange("p q (h x) -> p q h x", x=65)
            num = pvr[:, :, :, 0:HD]
            rcp = pool_o.tile([P, 2, 3], F32, tag="rcp")
            nc.vector.reciprocal(rcp[:], pvr[:, :, :, HD])
            dst = attn_nat[:, 2 * qc:2 * qc + 2, 3 * r:3 * r + 3, :]
            nc.vector.tensor_tensor(
                dst, num,
                rcp[:].unsqueeze(-1).broadcast_to((P, 2, 3, HD)),
                mybir.AluOpType.mult)

        # attn_nat -> attnT for this chunk's two q-tiles
        for qbl in range(2):
            qb = 2 * qc + qbl
            pjt = ps_pj.tile([P, 512], F32, tag="pj")
            bfv = pjt[:].bitcast(BF16).rearrange("p (n c) -> p n c", c=P)
            for e in range(DT):
                nc.tensor.transpose(
                    bfv[:, e],
                    attn_nat[:, qb, 2 * e:2 * e + 2, :].rearrange(
                        "p a b -> p (a b)"),
                    ident_sb[:])
            nc.vector.tensor_copy(
                attnT[:, :, qb * P:(qb + 1) * P],
                pjt[:].bitcast(BF16)[:, 0:6 * P].rearrange(
                    "p (d x) -> p d x", x=P))
        # out projection for this chunk's two s-tiles
        for qbl in range(2):
            st = 2 * qc + qbl
            ot = pool_o.tile([P, D], BF16, tag="ost")
            for n0, nw in ((0, 512), (512, 256)):
                po = ps_pj.tile([P, 512], F32, tag="pj")
                for dt in range(DT):
                    nc.tensor.matmul(
                        po[:, 0:nw],
                        attnT[:, dt, st * P:(st + 1) * P],
                        wo_sb[:, dt, n0:n0 + nw],
                        start=(dt == 0), stop=(dt == DT - 1))
                nc.vector.tensor_copy(ot[:, n0:n0 + nw], po[:, 0:nw])
            nc.sync.dma_start(
                out_d[st * P:(st + 1) * P, :], ot[:])


def build_module():
    nc = bacc.Bacc("TRN2", target_bir_lowering=False, debug=False)
    aps = {
        "xb": nc.dram_tensor("xb", [S, D], BF16, kind="ExternalInput").ap(),
        "wqk8": nc.dram_tensor("wqk8", [D, 2 * D], F8,
                               kind="ExternalInput").ap(),
        "wv": nc.dram_tensor("wv", [D, D], BF16, kind="ExternalInput").ap(),
        "wo": nc.dram_tensor("wo", [D, D], BF16, kind="ExternalInput").ap(),
        "identb": nc.dram_tensor("identb", [P, P], BF16,
                                 kind="ExternalInput").ap(),
        "trib4": nc.dram_tensor("trib4", [P, 4 * P], BF16,
                                kind="ExternalInput").ap(),
        "out": nc.dram_tensor("out", [S, D], BF16, kind="ExternalOutput").ap(),
    }
    from contextlib import ExitStack
    with tile.TileContext(nc) as tc, ExitStack() as ctx:
        build(ctx, tc, aps)
    nc.compile()
    return nc


def _perm_cols():
    cols = []
    for g in range(3):
        for s_ in range(2):
            for j in range(4):
                for t in range(32):
                    cols.append((4 * g + j) * HD + 32 * s_ + t)
    return cols


def kernel(hidden_states, Wqkv, bqkv, Wout, bout, _run_kwargs=None):
    hidden_states = np.asarray(hidden_states, dtype=np.float32)
    Wqkv = np.asarray(Wqkv, dtype=np.float32)
    bqkv = np.asarray(bqkv, dtype=np.float32)
    Wout = np.asarray(Wout, dtype=np.float32)
    bout = np.asarray(bout, dtype=np.float32)
    assert not np.any(bqkv), "nonzero qkv bias not supported by this kernel"

    nc = build_module()

    npbf = mybir.dt.np(BF16)
    npf8 = mybir.dt.np(F8)
    qcols = _perm_cols()
    perm = qcols + [D + c for c in qcols]
    wqk8 = (WS * Wqkv[:, perm]).astype(npf8)
    wv = Wqkv[:, 2 * D:3 * D].astype(npbf)
    wo = Wout.astype(npbf)
    identb = np.eye(P, dtype=np.float32).astype(npbf)
    tri = np.triu(np.ones((P, P), dtype=np.float32))
    on = np.ones((P, P), dtype=np.float32)
    trib4 = np.concatenate([tri, on, on, tri], axis=1).astype(npbf)
    in_maps = [
        {
            "xb": hidden_states[b].astype(npbf),
            "wqk8": wqk8,
            "wv": wv,
            "wo": wo,
            "identb": identb,
            "trib4": trib4,
        }
        for b in range(B)
    ]
    res = run_bass_kernel_spmd(nc, in_maps, core_ids=list(range(B)),
                               **(_run_kwargs or {}))
    out = np.stack([res.results[b]["out"].astype(np.float32)
                    for b in range(B)])
    if np.any(bout):
        out = out + bout
    kernel.last_results = res
    return out


# revision 19
# speedup vs baseline: 1.3492x; 1.0668x over previous
"""Multi-head causal self-attention (B=8, S=1024, D=768, H=12) on 8 TRN2
NeuronCores, data-parallel over the batch (one batch element per core).

Per-core pipeline (dtype-tiered for the TRN2 cost model):
  1. x arrives bf16; PE transposes build xT in bf16 (for V) and the PSUM is
     also converted to fp8 (for Q,K) on the scalar engine.
  2. Q,K projection runs fp8 DoubleRow (dt-pairs -> 256-deep contraction at
     0.5 cyc/row). Wqkv's Q,K columns are PERMUTED on the host so each
     128-row PSUM tile holds [4 heads x 32 hd-dims]; the evacuated fp8 qkT
     planes then feed DoubleRow scores directly: head h uses partitions
     32*(h%4).. with the two 32-dim slabs as the DoubleRow pair. Host column
     order also groups head-group-0 planes first so a split DMA unblocks the
     first scores early.
  3. Scores (fp8 DR, transposed layout [k,q]) -> exp on ACT (scale folds the
     1/sqrt(hd) and the fp8 weight scaling) -> bf16 expT -> causal mask
     multiply (one DVE op per head-chunk over a prebuilt [tri|1|1|tri]).
  4. PV runs in NATURAL orientation: out[q_part, head, hd+1] accumulated
     over k-tiles with v|ones moving operand; the ones column gives the
     softmax denominator per (q-partition, head) -> reciprocal + one
     broadcast multiply fuse normalization with the PSUM evacuation.
  5. attn_nat -> attnT via PE transposes; out-projection in bf16 with a
     telescoped accumulation (dt0-3 open early, dt4-5 after the last
     triple) so the post-softmax tail is short. bf16 result DMA'd out and
     upcast on the host. Attention runs q-chunk-major so out-projection
     overlaps later chunks' exp/PV; V projection and the second Q,K chunk
     are interleaved between head triples to keep PE fed without starving
     the exp stream.
"""

import sys

import numpy as np

for _p in ("/opt/trn_rl_repo", "/root/.axon_site/_ro/trn_rl_repo"):
    if _p not in sys.path:
        sys.path.append(_p)

import concourse.mybir as mybir  # noqa: E402
import concourse.tile as tile  # noqa: E402
from concourse import bacc  # noqa: E402
from concourse.bass_utils import run_bass_kernel_spmd  # noqa: E402

F32 = mybir.dt.float32
BF16 = mybir.dt.bfloat16
F8 = mybir.dt.float8e4

B, S, D = 8, 1024, 768
H, HD = 12, 64
P = 128
DT = 6            # 768 / 128 contraction tiles
ST = 8            # 1024 / 128 sequence tiles
QCW = 256         # q-chunk width for attention
NQC = S // QCW    # 4
WS = 128.0        # host scale baked into wqk8 (lifts W out of fp8 subnormals)
EXP_SCALE = 0.125 / (WS * WS)


def build(ctx, tc: tile.TileContext, aps: dict):
    nc = tc.nc
    xb_d, wqk_d, wv_d, wo_d, id_d, tri_d, out_d = (
        aps["xb"], aps["wqk8"], aps["wv"], aps["wo"],
        aps["identb"], aps["trib4"], aps["out"])

    pool_p = ctx.enter_context(tc.tile_pool(name="persist", bufs=1))
    pool_e = ctx.enter_context(tc.tile_pool(name="expT", bufs=16))
    pool_o = ctx.enter_context(tc.tile_pool(name="ostage", bufs=2))
    ps_sc = ctx.enter_context(tc.tile_pool(name="psSC", bufs=2, space="PSUM"))
    ps_u = ctx.enter_context(tc.tile_pool(name="psU", bufs=4, space="PSUM"))

    # ---- persistent SBUF ----
    x_b = pool_p.tile([P, ST, D], BF16, tag="xb")
    xT8 = pool_p.tile([P, DT, S], F8, tag="xT8")
    xTb = pool_p.tile([P, DT, S], BF16, tag="xTb")
    wqk_sb = pool_p.tile([P, DT, 2 * D], F8, tag="wqk")
    wv_sb = pool_p.tile([P, DT, D], BF16, tag="wv")
    wo_sb = pool_p.tile([P, DT, D], BF16, tag="wo")
    qkT = pool_p.tile([P, 2 * DT, S], F8, tag="qkT")  # plane 4g+{0,1}=q, +{2,3}=k
    v_buf = pool_p.tile([P, ST, H, HD + 1], BF16, tag="vbuf")
    attn_nat = pool_p.tile([P, ST, H, HD], BF16, tag="anat")
    attnT = pool_p.tile([P, DT, S], BF16, tag="attnT")
    ident_sb = pool_p.tile([P, P], BF16, tag="ident")
    tri4_sb = pool_p.tile([P, 4 * P], BF16, tag="tri4")
    scratch = pool_p.tile([1, 8], BF16, tag="scr")

    # ---- input DMAs (order = arrival order on the shared DMA device) ----
    xb_r = xb_d.rearrange("(t p) d -> p t d", p=P)
    wqk_r = wqk_d.rearrange("(t p) d -> p t d", p=P)
    nc.sync.dma_start(ident_sb[:], id_d)
    nc.sync.dma_start(x_b[:, 0:2], xb_r[:, 0:2])
    nc.sync.dma_start(wqk_sb[:, :, 0:512], wqk_r[:, :, 0:512])
    nc.sync.dma_start(tri4_sb[:], tri_d)
    nc.sync.dma_start(x_b[:, 2:4], xb_r[:, 2:4])
    nc.sync.dma_start(wqk_sb[:, :, 512:1536], wqk_r[:, :, 512:1536])
    nc.sync.dma_start(x_b[:, 4:8], xb_r[:, 4:8])
    nc.sync.dma_start(wv_sb[:], wv_d.rearrange("(t p) d -> p t d", p=P))
    nc.sync.dma_start(wo_sb[:], wo_d.rearrange("(t p) d -> p t d", p=P))

    # preload the exp table set while DMAs stream
    nc.scalar.activation(scratch[0:1, 0:1], ident_sb[0:1, 0:1],
                         mybir.ActivationFunctionType.Exp, scale=1.0)
    # ones columns of v_buf (softmax denominators ride the PV matmul)
    nc.gpsimd.memset(v_buf[:, :, :, HD], 1.0)

    # ---- bf16 transposes: xb -> xTb (DVE evac) + xT8 (fp8 convert, ACT) ----
    def tb_chunk(st_range):
        for st in st_range:
            pjt = ps_u.tile([P, 512], F32, tag="u")
            bfv = pjt[:].bitcast(BF16).rearrange("p (n c) -> p n c", c=P)
            for dt in range(DT):
                nc.tensor.transpose(
                    bfv[:, dt], x_b[:, st, dt * P:(dt + 1) * P], ident_sb[:])
            src = pjt[:].bitcast(BF16)[:, 0:6 * P].rearrange(
                "p (d x) -> p d x", x=P)
            nc.scalar.copy(xT8[:, :, st * P:(st + 1) * P], src)
            nc.vector.tensor_copy(xTb[:, :, st * P:(st + 1) * P], src)

    # ---- QK projection (fp8 DoubleRow over dt-pairs) ----
    def qk_chunk(c, planes, evac_act=False):
        for pl in planes:
            pq = ps_u.tile([P, 512], F32, tag="u")
            for i in range(3):
                nc.tensor.matmul(
                    pq[:],
                    wqk_sb[:, 2 * i:2 * i + 2, pl * P:(pl + 1) * P],
                    xT8[:, 2 * i:2 * i + 2, c * 512:(c + 1) * 512],
                    start=(i == 0), stop=(i == 2),
                    perf_mode=mybir.MatmulPerfMode.DoubleRow)
            dst = qkT[:, pl, c * 512:(c + 1) * 512]
            if evac_act:
                nc.scalar.copy(dst, pq[:])
            else:
                nc.vector.tensor_copy(dst, pq[:])

    # ---- V projection (bf16) ----
    def v_chunk(st_range):
        for st in st_range:
            for n0, nw in ((0, 512), (512, 256)):
                pv_ = ps_u.tile([P, 512], F32, tag="u")
                for dt in range(DT):
                    nc.tensor.matmul(
                        pv_[:, 0:nw],
                        xTb[:, dt, st * P:(st + 1) * P],
                        wv_sb[:, dt, n0:n0 + nw],
                        start=(dt == 0), stop=(dt == DT - 1))
                nc.vector.tensor_copy(
                    v_buf[:, st, n0 // HD:(n0 + nw) // HD, 0:HD],
                    pv_[:, 0:nw].rearrange("p (h x) -> p h x", x=HD))

    # critical path to the first exp: x st0-1 -> xT8 c0-lead -> qk group-0
    # planes (ACT evacs: DVE is busy with xTb) -> scores qc0
    tb_chunk(range(0, 4))
    qk_chunk(0, [0, 1, 2, 3], evac_act=True)
    tb_chunk(range(4, 8))
    qk_chunk(0, [4, 5, 6, 7, 8, 9, 10, 11])
    v_chunk(range(0, 2))

    # extra PE work interleaved between head-triples of each q-chunk, kept
    # off the scores->exp critical path
    fills = {
        (0, 0): lambda: v_chunk([2]),
        (0, 1): lambda: v_chunk([3]),
        (1, 0): lambda: (qk_chunk(1, [0, 1, 2, 3]), v_chunk([4])),
        (1, 1): lambda: (qk_chunk(1, [4, 5, 6, 7]), v_chunk([5])),
        (1, 2): lambda: qk_chunk(1, [8, 9, 10, 11]),
        (2, 0): lambda: v_chunk([6]),
        (2, 1): lambda: v_chunk([7]),
    }

    # ---- attention, q-chunk-major ----
    for qc in range(NQC):
        nlive = 2 * qc + 2
        exp_tiles = {}
        proj_open = {}
        for r in range(4):                   # head triples
            for h in range(3 * r, 3 * r + 3):
                g, j = h // 4, h % 4
                et = pool_e.tile([P, ST, QCW], BF16, tag="expT")
                exp_tiles[h] = et
                for t0 in range(0, nlive, 4):
                    t1 = min(t0 + 4, nlive)
                    sct = ps_sc.tile([P, 4, QCW], F32, tag="sc")
                    for kt in range(t0, t1):
                        sk = P if kt == 2 * qc + 1 else 0
                        nc.tensor.matmul(
                            sct[:, kt - t0, sk:QCW],
                            qkT[32 * j:32 * j + 32, 4 * g + 2:4 * g + 4,
                                kt * P:(kt + 1) * P],
                            qkT[32 * j:32 * j + 32, 4 * g:4 * g + 2,
                                qc * QCW + sk:(qc + 1) * QCW],
                            start=True, stop=True,
                            perf_mode=mybir.MatmulPerfMode.DoubleRow,
                            tile_position=(32 * j, 0))
                    nc.scalar.activation(
                        et[:, t0:t1, :], sct[:, 0:t1 - t0, :],
                        mybir.ActivationFunctionType.Exp, scale=EXP_SCALE)
                # causal mask: [tri|1|1|tri] over the two diagonal planes
                dm = et[:].rearrange("p a b -> p (a b)")[
                    :, 2 * qc * QCW:2 * qc * QCW + 512]
                nc.vector.tensor_tensor(dm, dm, tri4_sb[:],
                                        mybir.AluOpType.mult)
            # PV for this triple (natural orientation, ones col = denom)
            pvt = ps_u.tile([P, 512], F32, tag="u")
            pvq = pvt[:].rearrange("p (q c) -> p q c", c=QCW)
            for qbl in range(2):
                qb = 2 * qc + qbl
                for hl in range(3):
                    h = 3 * r + hl
                    for kt in range(qb + 1):
                        nc.tensor.matmul(
                            pvq[:, qbl, hl * 65:hl * 65 + 65],
                            exp_tiles[h][:, kt, qbl * P:(qbl + 1) * P],
                            v_buf[:, kt, h, :],
                            start=(kt == 0), stop=(kt == qb))
            # fused evacuate + normalize: recip of the denominators, then
            # one multiply with a broadcast AP
            pvr = pvq[:, :, 0:195].rearrange("p q (h x) -> p q h x", x=65)
            rcp = pool_o.tile([P, 2, 3], F32, tag="rcp")
            nc.vector.reciprocal(rcp[:], pvr[:, :, :, HD])
            dst = attn_nat[:, 2 * qc:2 * qc + 2, 3 * r:3 * r + 3, :]
            nc.vector.tensor_tensor(
                dst, pvr[:, :, :, 0:HD],
                rcp[:].unsqueeze(-1).broadcast_to((P, 2, 3, HD)),
                mybir.AluOpType.mult)

            if (qc, r) in fills:
                fills[(qc, r)]()

            if r == 2:
                # heads 0-8 done: transpose attnT planes dt0-3 and open the
                # first s-tile's out-projection accumulation over dt0-3
                trt = ps_u.tile([P, 512], F32, tag="u")
                trv = trt[:].bitcast(BF16).rearrange("p (n c) -> p n c", c=P)
                for qbl in range(2):
                    qb = 2 * qc + qbl
                    for e in range(4):
                        nc.tensor.transpose(
                            trv[:, qbl * 4 + e],
                            attn_nat[:, qb, 2 * e:2 * e + 2, :].rearrange(
                                "p a b -> p (a b)"),
                            ident_sb[:])
                for qbl in range(2):
                    nc.vector.tensor_copy(
                        attnT[:, 0:4, (2 * qc + qbl) * P:(2 * qc + qbl + 1) * P],
                        trt[:].bitcast(BF16)[:, qbl * 512:qbl * 512 + 512]
                        .rearrange("p (d x) -> p d x", x=P))
                st0 = 2 * qc
                for n0, nw in ((0, 512), (512, 256)):
                    po = ps_u.tile([P, 512], F32, tag="u")
                    proj_open[n0] = po
                    for dt in range(4):
                        nc.tensor.matmul(
                            po[:, 0:nw],
                            attnT[:, dt, st0 * P:(st0 + 1) * P],
                            wo_sb[:, dt, n0:n0 + nw],
                            start=(dt == 0), stop=False)

        # tail: transpose dt4-5, close the open projection, do s-tile 1
        trt = ps_u.tile([P, 512], F32, tag="u")
        trv = trt[:].bitcast(BF16).rearrange("p (n c) -> p n c", c=P)
        for qbl in range(2):
            qb = 2 * qc + qbl
            for e in (4, 5):
                nc.tensor.transpose(
                    trv[:, qbl * 2 + (e - 4)],
                    attn_nat[:, qb, 2 * e:2 * e + 2, :].rearrange(
                        "p a b -> p (a b)"),
                    ident_sb[:])
        for qbl in range(2):
            nc.vector.tensor_copy(
                attnT[:, 4:6, (2 * qc + qbl) * P:(2 * qc + qbl + 1) * P],
                trt[:].bitcast(BF16)[:, qbl * 256:qbl * 256 + 256]
                .rearrange("p (d x) -> p d x", x=P))

        st0, st1 = 2 * qc, 2 * qc + 1
        ot0 = pool_o.tile([P, D], BF16, tag="ost")
        for n0, nw in ((0, 512), (512, 256)):
            po = proj_open[n0]
            for dt in (4, 5):
                nc.tensor.matmul(
                    po[:, 0:nw],
                    attnT[:, dt, st0 * P:(st0 + 1) * P],
                    wo_sb[:, dt, n0:n0 + nw],
                    start=False, stop=(dt == 5))
            nc.vector.tensor_copy(ot0[:, n0:n0 + nw], po[:, 0:nw])
        nc.sync.dma_start(out_d[st0 * P:(st0 + 1) * P, :], ot0[:])

        ot1 = pool_o.tile([P, D], BF16, tag="ost")
        for n0, nw in ((0, 512), (512, 256)):
            po = ps_u.tile([P, 512], F32, tag="u")
            for dt in range(DT):
                nc.tensor.matmul(
                    po[:, 0:nw],
                    attnT[:, dt, st1 * P:(st1 + 1) * P],
                    wo_sb[:, dt, n0:n0 + nw],
                    start=(dt == 0), stop=(dt == DT - 1))
            nc.vector.tensor_copy(ot1[:, n0:n0 + nw], po[:, 0:nw])
        nc.sync.dma_start(out_d[st1 * P:(st1 + 1) * P, :], ot1[:])


def build_module():
    nc = bacc.Bacc("TRN2", target_bir_lowering=False, debug=False)
    aps = {
        "xb": nc.dram_tensor("xb", [S, D], BF16, kind="ExternalInput").ap(),
        "wqk8": nc.dram_tensor("wqk8", [D, 2 * D], F8,
                               kind="ExternalInput").ap(),
        "wv": nc.dram_tensor("wv", [D, D], BF16, kind="ExternalInput").ap(),
        "wo": nc.dram_tensor("wo", [D, D], BF16, kind="ExternalInput").ap(),
        "identb": nc.dram_tensor("identb", [P, P], BF16,
                                 kind="ExternalInput").ap(),
        "trib4": nc.dram_tensor("trib4", [P, 4 * P], BF16,
                                kind="ExternalInput").ap(),
        "out": nc.dram_tensor("out", [S, D], BF16, kind="ExternalOutput").ap(),
    }
    from contextlib import ExitStack
    with tile.TileContext(nc) as tc, ExitStack() as ctx:
        build(ctx, tc, aps)
    nc.compile()
    return nc


def _perm_cols():
    """Column order for wqk8: plane 4g+{0,1} = q slabs, 4g+{2,3} = k slabs;
    within a plane, 4 heads x 32 dims."""
    cols = []
    for g in range(3):
        for qk in range(2):
            for s_ in range(2):
                for j in range(4):
                    for t in range(32):
                        cols.append(qk * D + (4 * g + j) * HD + 32 * s_ + t)
    return cols


def kernel(hidden_states, Wqkv, bqkv, Wout, bout, _run_kwargs=None):
    hidden_states = np.asarray(hidden_states, dtype=np.float32)
    Wqkv = np.asarray(Wqkv, dtype=np.float32)
    bqkv = np.asarray(bqkv, dtype=np.float32)
    Wout = np.asarray(Wout, dtype=np.float32)
    bout = np.asarray(bout, dtype=np.float32)
    assert not np.any(bqkv), "nonzero qkv bias not supported by this kernel"

    nc = build_module()

    npbf = mybir.dt.np(BF16)
    npf8 = mybir.dt.np(F8)
    wqk8 = (WS * Wqkv[:, _perm_cols()]).astype(npf8)
    wv = Wqkv[:, 2 * D:3 * D].astype(npbf)
    wo = Wout.astype(npbf)
    identb = np.eye(P, dtype=np.float32).astype(npbf)
    tri = np.triu(np.ones((P, P), dtype=np.float32))
    on = np.ones((P, P), dtype=np.float32)
    trib4 = np.concatenate([tri, on, on, tri], axis=1).astype(npbf)
    in_maps = [
        {
            "xb": hidden_states[b].astype(npbf),
            "wqk8": wqk8,
            "wv": wv,
            "wo": wo,
            "identb": identb,
            "trib4": trib4,
        }
        for b in range(B)
    ]
    res = run_bass_kernel_spmd(nc, in_maps, core_ids=list(range(B)),
                               **(_run_kwargs or {}))
    out = np.stack([res.results[b]["out"].astype(np.float32)
                    for b in range(B)])
    if np.any(bout):
        out = out + bout
    kernel.last_results = res
    return out


# revision 24
# speedup vs baseline: 1.5023x; 1.1135x over previous
"""Multi-head causal self-attention (B=8, S=1024, D=768, H=12) on 8 TRN2
NeuronCores, data-parallel over the batch (one batch element per core).

Per-core pipeline (dtype-tiered for the TRN2 cost model):
  1. x arrives bf16; PE transposes build xTb. xT8 (fp8 for Q,K) is converted
     on ACT for the first chunk (fast lead-in) and on gpsimd for the rest.
  2. Q,K projection runs fp8 DoubleRow (dt-pairs -> 256-deep contraction at
     0.5 cyc/row). Wqkv's Q,K columns are PERMUTED on the host so each
     128-row PSUM tile holds [4 heads x 32 hd-dims]; the evacuated fp8 qkT
     planes feed DoubleRow scores directly: head h uses partitions
     32*(h%4).. with the two 32-dim slabs as the DoubleRow pair. Host column
     order groups head-group-0 planes first so a split DMA unblocks the
     first scores early.
  3. Scores (fp8 DR, transposed layout [k,q]) -> exp on ACT (scale folds the
     1/sqrt(hd) and the fp8 weight scaling) -> bf16 expT -> causal mask
     multiply on gpsimd (both diagonal blocks in one strided AP).
  4. PV runs in NATURAL orientation per head-PAIR: out[q_part, 2h, hd+1]
     accumulated over k-tiles with v|ones moving operand; the ones column
     gives the softmax denominator per (q-partition, head) -> reciprocal +
     one broadcast multiply fuse normalization with the PSUM evacuation.
  5. attn_nat -> attnT via PE transposes; out-projection in bf16 with a
     telescoped accumulation (dt0-3 open after head-pair 3, dt4-5 at the
     end) so the post-softmax tail is short. bf16 result DMA'd out and
     upcast on the host. Attention runs q-chunk-major in order [1,2,3,0]
     (qc1 needs only the first projection chunk; qc0 finishes fastest);
     V projection and the second Q,K chunk are emitted at LOW priority so
     the scheduler backfills PE with them without starving the exp stream.
"""

import sys

import numpy as np

for _p in ("/opt/trn_rl_repo", "/root/.axon_site/_ro/trn_rl_repo"):
    if _p not in sys.path:
        sys.path.append(_p)

import concourse.mybir as mybir  # noqa: E402
import concourse.tile as tile  # noqa: E402
from concourse import bacc  # noqa: E402
from concourse.bass_utils import run_bass_kernel_spmd  # noqa: E402

F32 = mybir.dt.float32
BF16 = mybir.dt.bfloat16
F8 = mybir.dt.float8e4

B, S, D = 8, 1024, 768
H, HD = 12, 64
P = 128
DT = 6            # 768 / 128 contraction tiles
ST = 8            # 1024 / 128 sequence tiles
QCW = 256         # q-chunk width for attention
NQC = S // QCW    # 4
WS = 128.0        # host scale baked into wqk8 (lifts W out of fp8 subnormals)
EXP_SCALE = 0.125 / (WS * WS)
LOWPRI = -500000  # emission-priority demotion for backfill work


def build(ctx, tc: tile.TileContext, aps: dict):
    nc = tc.nc
    xb_d, wqk_d, wv_d, wo_d, id_d, tri_d, out_d = (
        aps["xb"], aps["wqk8"], aps["wv"], aps["wo"],
        aps["identb"], aps["trib4"], aps["out"])

    pool_p = ctx.enter_context(tc.tile_pool(name="persist", bufs=1))
    pool_e = ctx.enter_context(tc.tile_pool(name="expT", bufs=16))
    pool_o = ctx.enter_context(tc.tile_pool(name="ostage", bufs=2))
    ps_sc = ctx.enter_context(tc.tile_pool(name="psSC", bufs=2, space="PSUM"))
    ps_u = ctx.enter_context(tc.tile_pool(name="psU", bufs=4, space="PSUM"))

    # ---- persistent SBUF ----
    x_b = pool_p.tile([P, ST, D], BF16, tag="xb")
    xT8 = pool_p.tile([P, DT, S], F8, tag="xT8")
    xTb = pool_p.tile([P, DT, S], BF16, tag="xTb")
    wqk_sb = pool_p.tile([P, DT, 2 * D], F8, tag="wqk")
    wv_sb = pool_p.tile([P, DT, D], BF16, tag="wv")
    wo_sb = pool_p.tile([P, DT, D], BF16, tag="wo")
    qkT = pool_p.tile([P, 2 * DT, S], F8, tag="qkT")  # plane 4g+{0,1}=q, +{2,3}=k
    v_buf = pool_p.tile([P, ST, H, HD + 1], BF16, tag="vbuf")
    attn_nat = pool_p.tile([P, ST, H, HD], BF16, tag="anat")
    attnT = pool_p.tile([P, DT, S], BF16, tag="attnT")
    ident_sb = pool_p.tile([P, P], BF16, tag="ident")
    tri4_sb = pool_p.tile([P, 4 * P], BF16, tag="tri4")
    scratch = pool_p.tile([1, 8], BF16, tag="scr")

    # ---- input DMAs (order = arrival order on the shared DMA device) ----
    xb_r = xb_d.rearrange("(t p) d -> p t d", p=P)
    wqk_r = wqk_d.rearrange("(t p) d -> p t d", p=P)
    nc.sync.dma_start(ident_sb[:], id_d)
    nc.sync.dma_start(x_b[:, 0:2], xb_r[:, 0:2])
    nc.sync.dma_start(wqk_sb[:, :, 0:512], wqk_r[:, :, 0:512])
    nc.sync.dma_start(tri4_sb[:], tri_d)
    nc.sync.dma_start(x_b[:, 2:4], xb_r[:, 2:4])
    nc.sync.dma_start(wqk_sb[:, :, 512:1536], wqk_r[:, :, 512:1536])
    nc.sync.dma_start(x_b[:, 4:8], xb_r[:, 4:8])
    nc.sync.dma_start(wv_sb[:], wv_d.rearrange("(t p) d -> p t d", p=P))
    nc.sync.dma_start(wo_sb[:], wo_d.rearrange("(t p) d -> p t d", p=P))

    # preload the exp table set while DMAs stream
    nc.scalar.activation(scratch[0:1, 0:1], ident_sb[0:1, 0:1],
                         mybir.ActivationFunctionType.Exp, scale=1.0)
    # ones columns of v_buf (softmax denominators ride the PV matmul)
    nc.gpsimd.memset(v_buf[:, :, :, HD], 1.0)

    # ---- bf16 transposes: xb -> xTb (DVE evac); xT8 on ACT (first chunk,
    # fast lead-in) or gpsimd (rest, keeps ACT free for exp) ----
    def tb_chunk(st_range):
        for st in st_range:
            pjt = ps_u.tile([P, 512], F32, tag="u")
            bfv = pjt[:].bitcast(BF16).rearrange("p (n c) -> p n c", c=P)
            for dt in range(DT):
                nc.tensor.transpose(
                    bfv[:, dt], x_b[:, st, dt * P:(dt + 1) * P], ident_sb[:])
            src = pjt[:].bitcast(BF16)[:, 0:6 * P].rearrange(
                "p (d x) -> p d x", x=P)
            dst8 = xT8[:, :, st * P:(st + 1) * P]
            dstb = xTb[:, :, st * P:(st + 1) * P]
            if st < 4:
                nc.scalar.copy(dst8, src)
                nc.vector.tensor_copy(dstb, src)
            else:
                nc.vector.tensor_copy(dstb, src)
                nc.gpsimd.tensor_copy(dst8, dstb)

    # ---- QK projection (fp8 DoubleRow over dt-pairs) ----
    def qk_chunk(c, planes, evac_act=False):
        for pl in planes:
            pq = ps_u.tile([P, 512], F32, tag="u")
            for i in range(3):
                nc.tensor.matmul(
                    pq[:],
                    wqk_sb[:, 2 * i:2 * i + 2, pl * P:(pl + 1) * P],
                    xT8[:, 2 * i:2 * i + 2, c * 512:(c + 1) * 512],
                    start=(i == 0), stop=(i == 2),
                    perf_mode=mybir.MatmulPerfMode.DoubleRow)
            dst = qkT[:, pl, c * 512:(c + 1) * 512]
            if evac_act:
                nc.scalar.copy(dst, pq[:])
            else:
                nc.vector.tensor_copy(dst, pq[:])

    # ---- V projection (bf16) ----
    def v_chunk(st_range):
        for st in st_range:
            for n0, nw in ((0, 512), (512, 256)):
                pv_ = ps_u.tile([P, 512], F32, tag="u")
                for dt in range(DT):
                    nc.tensor.matmul(
                        pv_[:, 0:nw],
                        xTb[:, dt, st * P:(st + 1) * P],
                        wv_sb[:, dt, n0:n0 + nw],
                        start=(dt == 0), stop=(dt == DT - 1))
                nc.vector.tensor_copy(
                    v_buf[:, st, n0 // HD:(n0 + nw) // HD, 0:HD],
                    pv_[:, 0:nw].rearrange("p (h x) -> p h x", x=HD))

    # critical path to the first exp: x st0-1 -> xT8 c0-lead -> qk group-0
    # planes (ACT evacs: DVE is busy with xTb) -> scores qc1
    tb_chunk(range(0, 4))
    qk_chunk(0, [0, 1, 2, 3], evac_act=True)
    tb_chunk(range(4, 8))
    qk_chunk(0, [4, 5, 6, 7, 8, 9, 10, 11])
    with tc.high_priority(offset=LOWPRI):
        v_chunk(range(0, 4))

    # backfill PE work per (qc, pair) — emitted at LOW priority so the
    # scores->exp chain always wins ties on the Tensor engine
    fills = {
        (1, 0): lambda: qk_chunk(1, [0, 1, 2, 3]),
        (1, 1): lambda: qk_chunk(1, [4, 5, 6, 7]),
        (1, 2): lambda: qk_chunk(1, [8, 9, 10, 11]),
        (1, 3): lambda: v_chunk([4]),
        (2, 0): lambda: v_chunk([5]),
        (2, 2): lambda: v_chunk([6]),
        (3, 0): lambda: v_chunk([7]),
    }

    # two diagonal [128,128] blocks (and matching [tri|tri]) as one AP
    tri2 = tri4_sb[:].rearrange("p (b c) -> p b c", c=P)[:, 0::3, :]

    # ---- attention, q-chunk-major, head-pair groups ----
    for qc in (1, 2, 3, 0):
        nlive = 2 * qc + 2
        last = qc == 0
        proj_open = {}
        for r in range(6):                   # head pairs
            ets = {}
            if nlive == 2:
                # both heads' scores fit one sc quad: single exp call
                et = pool_e.tile([P, ST, QCW], BF16, tag="expT")
                sct = ps_sc.tile([P, 4, QCW], F32, tag="sc")
                for hl in range(2):
                    h, (g, j) = 2 * r + hl, divmod(2 * r + hl, 4)
                    ets[hl] = (et, 2 * hl)
                    for kt in range(2):
                        sk = P if kt == 1 else 0
                        nc.tensor.matmul(
                            sct[:, 2 * hl + kt, sk:QCW],
                            qkT[32 * j:32 * j + 32, 4 * g + 2:4 * g + 4,
                                kt * P:(kt + 1) * P],
                            qkT[32 * j:32 * j + 32, 4 * g:4 * g + 2,
                                qc * QCW + sk:(qc + 1) * QCW],
                            start=True, stop=True,
                            perf_mode=mybir.MatmulPerfMode.DoubleRow,
                            tile_position=(32 * j, 0))
                nc.scalar.activation(et[:, 0:4, :], sct[:],
                                     mybir.ActivationFunctionType.Exp,
                                     scale=EXP_SCALE)
                for hl in range(2):
                    dmo = 2 * hl * QCW
                    dm = et[:].rearrange("p a b -> p (a b)")[
                        :, dmo:dmo + 512].rearrange(
                        "p (b c) -> p b c", c=P)[:, 0::3, :]
                    nc.gpsimd.tensor_tensor(dm, dm, tri2,
                                            mybir.AluOpType.mult)
            else:
                for hl in range(2):
                    h, (g, j) = 2 * r + hl, divmod(2 * r + hl, 4)
                    et = pool_e.tile([P, ST, QCW], BF16, tag="expT")
                    ets[hl] = (et, 0)
                    for t0 in range(0, nlive, 4):
                        t1 = min(t0 + 4, nlive)
                        sct = ps_sc.tile([P, 4, QCW], F32, tag="sc")
                        for kt in range(t0, t1):
                            sk = P if kt == 2 * qc + 1 else 0
                            nc.tensor.matmul(
                                sct[:, kt - t0, sk:QCW],
                                qkT[32 * j:32 * j + 32, 4 * g + 2:4 * g + 4,
                                    kt * P:(kt + 1) * P],
                                qkT[32 * j:32 * j + 32, 4 * g:4 * g + 2,
                                    qc * QCW + sk:(qc + 1) * QCW],
                                start=True, stop=True,
                                perf_mode=mybir.MatmulPerfMode.DoubleRow,
                                tile_position=(32 * j, 0))
                        nc.scalar.activation(
                            et[:, t0:t1, :], sct[:, 0:t1 - t0, :],
                            mybir.ActivationFunctionType.Exp, scale=EXP_SCALE)
                    dmo = 2 * qc * QCW
                    dm = et[:].rearrange("p a b -> p (a b)")[
                        :, dmo:dmo + 512].rearrange(
                        "p (b c) -> p b c", c=P)[:, 0::3, :]
                    nc.gpsimd.tensor_tensor(dm, dm, tri2,
                                            mybir.AluOpType.mult)

            # PV for this pair (natural orientation, ones col = denom)
            pvt = ps_u.tile([P, 512], F32, tag="u")
            pvq = pvt[:].rearrange("p (q c) -> p q c", c=QCW)
            for qbl in range(2):
                qb = 2 * qc + qbl
                for hl in range(2):
                    h = 2 * r + hl
                    et, off = ets[hl]
                    for kt in range(qb + 1):
                        nc.tensor.matmul(
                            pvq[:, qbl, hl * 65:hl * 65 + 65],
                            et[:, off + kt, qbl * P:(qbl + 1) * P],
                            v_buf[:, kt, h, :],
                            start=(kt == 0), stop=(kt == qb))
            # fused evacuate + normalize
            pvr = pvq[:, :, 0:130].rearrange("p q (h x) -> p q h x", x=65)
            rcp = pool_o.tile([P, 2, 2], F32, tag="rcp")
            nc.vector.reciprocal(rcp[:], pvr[:, :, :, HD])
            dst = attn_nat[:, 2 * qc:2 * qc + 2, 2 * r:2 * r + 2, :]
            nc.vector.tensor_tensor(
                dst, pvr[:, :, :, 0:HD],
                rcp[:].unsqueeze(-1).broadcast_to((P, 2, 2, HD)),
                mybir.AluOpType.mult)

            if (qc, r) in fills:
                with tc.high_priority(offset=LOWPRI):
                    fills[(qc, r)]()

            if r == 3:
                # heads 0-7 done: transpose attnT planes dt0-3 and open the
                # first s-tile's out-projection accumulation over dt0-3
                trt = ps_u.tile([P, 512], F32, tag="u")
                trv = trt[:].bitcast(BF16).rearrange("p (n c) -> p n c", c=P)
                for qbl in range(2):
                    qb = 2 * qc + qbl
                    for e in range(4):
                        nc.tensor.transpose(
                            trv[:, qbl * 4 + e],
                            attn_nat[:, qb, 2 * e:2 * e + 2, :].rearrange(
                                "p a b -> p (a b)"),
                            ident_sb[:])
                for qbl in range(2):
                    nc.vector.tensor_copy(
                        attnT[:, 0:4, (2 * qc + qbl) * P:(2 * qc + qbl + 1) * P],
                        trt[:].bitcast(BF16)[:, qbl * 512:qbl * 512 + 512]
                        .rearrange("p (d x) -> p d x", x=P))
                st0 = 2 * qc
                for n0, nw in ((0, 512), (512, 256)):
                    po = ps_u.tile([P, 512], F32, tag="u")
                    proj_open[n0] = po
                    for dt in range(4):
                        nc.tensor.matmul(
                            po[:, 0:nw],
                            attnT[:, dt, st0 * P:(st0 + 1) * P],
                            wo_sb[:, dt, n0:n0 + nw],
                            start=(dt == 0), stop=False)

        # tail: transpose dt4-5, close the open projection, do s-tile 1
        trt = ps_u.tile([P, 512], F32, tag="u")
        trv = trt[:].bitcast(BF16).rearrange("p (n c) -> p n c", c=P)
        for qbl in range(2):
            qb = 2 * qc + qbl
            for e in (4, 5):
                nc.tensor.transpose(
                    trv[:, qbl * 2 + (e - 4)],
                    attn_nat[:, qb, 2 * e:2 * e + 2, :].rearrange(
                        "p a b -> p (a b)"),
                    ident_sb[:])
        for qbl in range(2):
            nc.vector.tensor_copy(
                attnT[:, 4:6, (2 * qc + qbl) * P:(2 * qc + qbl + 1) * P],
                trt[:].bitcast(BF16)[:, qbl * 256:qbl * 256 + 256]
                .rearrange("p (d x) -> p d x", x=P))

        st0, st1 = 2 * qc, 2 * qc + 1
        ot0 = pool_o.tile([P, D], BF16, tag="ost")
        for n0, nw in ((0, 512), (512, 256)):
            po = proj_open[n0]
            for dt in (4, 5):
                nc.tensor.matmul(
                    po[:, 0:nw],
                    attnT[:, dt, st0 * P:(st0 + 1) * P],
                    wo_sb[:, dt, n0:n0 + nw],
                    start=False, stop=(dt == 5))
            if last:
                nc.scalar.copy(ot0[:, n0:n0 + nw], po[:, 0:nw])
                nc.sync.dma_start(out_d[st0 * P:(st0 + 1) * P, n0:n0 + nw],
                                  ot0[:, n0:n0 + nw])
            else:
                nc.vector.tensor_copy(ot0[:, n0:n0 + nw], po[:, 0:nw])
        if not last:
            nc.sync.dma_start(out_d[st0 * P:(st0 + 1) * P, :], ot0[:])

        ot1 = pool_o.tile([P, D], BF16, tag="ost")
        for n0, nw in ((0, 512), (512, 256)):
            po = ps_u.tile([P, 512], F32, tag="u")
            for dt in range(DT):
                nc.tensor.matmul(
                    po[:, 0:nw],
                    attnT[:, dt, st1 * P:(st1 + 1) * P],
                    wo_sb[:, dt, n0:n0 + nw],
                    start=(dt == 0), stop=(dt == DT - 1))
            if last:
                nc.scalar.copy(ot1[:, n0:n0 + nw], po[:, 0:nw])
                nc.sync.dma_start(out_d[st1 * P:(st1 + 1) * P, n0:n0 + nw],
                                  ot1[:, n0:n0 + nw])
            else:
                nc.vector.tensor_copy(ot1[:, n0:n0 + nw], po[:, 0:nw])
        if not last:
            nc.sync.dma_start(out_d[st1 * P:(st1 + 1) * P, :], ot1[:])


def build_module():
    nc = bacc.Bacc("TRN2", target_bir_lowering=False, debug=False)
    aps = {
        "xb": nc.dram_tensor("xb", [S, D], BF16, kind="ExternalInput").ap(),
        "wqk8": nc.dram_tensor("wqk8", [D, 2 * D], F8,
                               kind="ExternalInput").ap(),
        "wv": nc.dram_tensor("wv", [D, D], BF16, kind="ExternalInput").ap(),
        "wo": nc.dram_tensor("wo", [D, D], BF16, kind="ExternalInput").ap(),
        "identb": nc.dram_tensor("identb", [P, P], BF16,
                                 kind="ExternalInput").ap(),
        "trib4": nc.dram_tensor("trib4", [P, 4 * P], BF16,
                                kind="ExternalInput").ap(),
        "out": nc.dram_tensor("out", [S, D], BF16, kind="ExternalOutput").ap(),
    }
    from contextlib import ExitStack
    with tile.TileContext(nc) as tc, ExitStack() as ctx:
        build(ctx, tc, aps)
    nc.compile()
    return nc


def _perm_cols():
    """Column order for wqk8: plane 4g+{0,1} = q slabs, 4g+{2,3} = k slabs;
    within a plane, 4 heads x 32 dims."""
    cols = []
    for g in range(3):
        for qk in range(2):
            for s_ in range(2):
                for j in range(4):
                    for t in range(32):
                        cols.append(qk * D + (4 * g + j) * HD + 32 * s_ + t)
    return cols


def kernel(hidden_states, Wqkv, bqkv, Wout, bout, _run_kwargs=None):
    hidden_states = np.asarray(hidden_states, dtype=np.float32)
    Wqkv = np.asarray(Wqkv, dtype=np.float32)
    bqkv = np.asarray(bqkv, dtype=np.float32)
    Wout = np.asarray(Wout, dtype=np.float32)
    bout = np.asarray(bout, dtype=np.float32)
    assert not np.any(bqkv), "nonzero qkv bias not supported by this kernel"

    nc = build_module()

    npbf = mybir.dt.np(BF16)
    npf8 = mybir.dt.np(F8)
    wqk8 = (WS * Wqkv[:, _perm_cols()]).astype(npf8)
    wv = Wqkv[:, 2 * D:3 * D].astype(npbf)
    wo = Wout.astype(npbf)
    identb = np.eye(P, dtype=np.float32).astype(npbf)
    tri = np.triu(np.ones((P, P), dtype=np.float32))
    on = np.ones((P, P), dtype=np.float32)
    trib4 = np.concatenate([tri, on, on, tri], axis=1).astype(npbf)
    in_maps = [
        {
            "xb": hidden_states[b].astype(npbf),
            "wqk8": wqk8,
            "wv": wv,
            "wo": wo,
            "identb": identb,
            "trib4": trib4,
        }
        for b in range(B)
    ]
    res = run_bass_kernel_spmd(nc, in_maps, core_ids=list(range(B)),
                               **(_run_kwargs or {}))
    out = np.stack([res.results[b]["out"].astype(np.float32)
                    for b in range(B)])
    if np.any(bout):
        out = out + bout
    kernel.last_results = res
    return out
